# revision 8
# baseline (speedup 1.0000x reference)
"""Weighted-BCE + masked-MSE loss on 8 Trainium2 cores (pure data parallel).

Math (t in {0,1} exactly): let q = |1 - p - t|  (= p when t=1, 1-p when t=0).
  ln(q^2) = 2*ln(q), so one Square+Ln chain replaces both BCE logs:
    class_sum = -[w1*S_tlq + w0*(S_lq - S_tlq)] / 2,
      S_lq  = sum ln(q^2)   (free via ACT accum),
      S_tlq = sum t*ln(q^2) (one DVE dot)
  masked sq: sum (1-t)*dd^2 = S_sq - S_tsq,  dd = ro - rt
  cnt_zeros = N - S_t  (PE matmul ones.T @ t)
Each core reduces its shard to 5 scalars; host combines and applies weights.

Engine mix per tile (DMA is the bottleneck; everything else has slack):
  DMA : p,t,rt dispatched on SP HWDGE; ro on ACT HWDGE
  Pool: s = p + t (its one op)
  ACT : u2 = Square(1-s); lq2 = Ln(u2) [+accum S_lq]; sq = Square(dd) [+accum S_sq]
  DVE : dd = ro - rt; two tensor_tensor_reduce dots (t.lq2, t.sq)
  PE  : count = ones.T @ t accumulated in PSUM; final partition reduce
"""

import os
import sys

for _p in ("/opt/trn_rl_repo", "/root/.axon_site/_ro/trn_rl_repo"):
    if os.path.isdir(_p) and _p not in sys.path:
        sys.path.insert(0, _p)

import numpy as np

import concourse.bacc as bacc
import concourse.mybir as mybir
from concourse import tile
from concourse.bass_utils import run_bass_kernel_spmd

N = 16777216
NCORES = 8
NSHARD = N // NCORES  # 2097152
P = 128
F = 2048
NT = NSHARD // (P * F)  # 8

_F32 = mybir.dt.float32

LAST_RESULTS = None  # test harness peeks at exec_time_ns / trace path


def _build_nc():
    AF = mybir.ActivationFunctionType
    OP = mybir.AluOpType
    AX = mybir.AxisListType

    nc = bacc.Bacc(
        "TRN2", target_bir_lowering=False, debug=False, num_devices=NCORES
    )
    p_d = nc.dram_tensor("p", [NT, P, F], _F32, kind="ExternalInput")
    t_d = nc.dram_tensor("t", [NT, P, F], _F32, kind="ExternalInput")
    ro_d = nc.dram_tensor("ro", [NT, P, F], _F32, kind="ExternalInput")
    rt_d = nc.dram_tensor("rt", [NT, P, F], _F32, kind="ExternalInput")
    out_d = nc.dram_tensor("out", [1, 5], _F32, kind="ExternalOutput")

    with tile.TileContext(nc) as tc:
        with (
            tc.tile_pool(name="io", bufs=3) as io,
            tc.tile_pool(name="work", bufs=2) as work,
            tc.tile_pool(name="stats", bufs=1) as stats,
            tc.tile_pool(name="psum", bufs=1, space="PSUM") as psum,
        ):
            acc_tlq = stats.tile([P, NT], _F32)  # sum t*ln(q^2) per tile col
            acc_lq = stats.tile([P, NT], _F32)  # sum ln(q^2)
            acc_sq = stats.tile([P, NT], _F32)  # sum (ro-rt)^2
            acc_tsq = stats.tile([P, NT], _F32)  # sum t*(ro-rt)^2

            ones_f = stats.tile([P, 1], _F32)
            nc.vector.memset(ones_f[:], 1.0)
            junk = stats.tile([P, F], _F32)  # dead dot output, reused

            psum_cnt = psum.tile([1, 512], _F32)
            NCHUNK = F // 512

            for i in range(NT):
                tp = io.tile([P, F], _F32, tag="p")
                tt = io.tile([P, F], _F32, tag="t")
                tro = io.tile([P, F], _F32, tag="ro")
                trt = io.tile([P, F], _F32, tag="rt")
                nc.sync.dma_start(tp[:], p_d[i, :, :])
                nc.sync.dma_start(tt[:], t_d[i, :, :])
                nc.sync.dma_start(tro[:], ro_d[i, :, :])
                nc.sync.dma_start(trt[:], rt_d[i, :, :])

                # Pool: s = p + t (its one op)
                s = work.tile([P, F], _F32, tag="s")
                nc.gpsimd.tensor_add(s[:], tp[:], tt[:])

                # ACT: u2 = (1-s)^2 = q^2 ; lq2 = ln(q^2) with free accum
                u2 = work.tile([P, F], _F32, tag="u2")
                nc.scalar.activation(u2[:], s[:], AF.Square, bias=1.0, scale=-1.0)
                lq2 = work.tile([P, F], _F32, tag="lq2")
                nc.scalar.activation(
                    lq2[:], u2[:], AF.Ln, accum_out=acc_lq[:, i : i + 1]
                )

                # DVE: dd = ro - rt; dot t.lq2
                dd = work.tile([P, F], _F32, tag="dd")
                nc.vector.tensor_sub(dd[:], tro[:], trt[:])
                nc.vector.scalar_tensor_tensor(
                    junk[:, 0:1].broadcast_to([P, F]), tt[:], 1.0, lq2[:],
                    OP.mult, OP.mult, accum_out=acc_tlq[:, i : i + 1],
                )

                # ACT: sq = dd^2 with free accum; DVE: dot t.sq
                sq = work.tile([P, F], _F32, tag="sq")
                nc.scalar.activation(
                    sq[:], dd[:], AF.Square, accum_out=acc_sq[:, i : i + 1]
                )
                nc.vector.scalar_tensor_tensor(
                    junk[:, 0:1].broadcast_to([P, F]), tt[:], 1.0, sq[:],
                    OP.mult, OP.mult, accum_out=acc_tsq[:, i : i + 1],
                )

                # PE: accumulate column-sums of t into psum_cnt
                for c in range(NCHUNK):
                    nc.tensor.matmul(
                        psum_cnt[0:1, :],
                        ones_f[:, 0:1],
                        tt[:, c * 512 : (c + 1) * 512],
                        start=(i == 0 and c == 0),
                        stop=(i == NT - 1 and c == NCHUNK - 1),
                    )

            # Fold per-tile partials into out[1,5]
            red = stats.tile([P, 4], _F32)
            for j, acc in enumerate((acc_tlq, acc_lq, acc_sq, acc_tsq)):
                nc.vector.tensor_reduce(red[:, j : j + 1], acc[:], AX.X, OP.add)
            psum_fin = psum.tile([1, 4], _F32)
            nc.tensor.matmul(
                psum_fin[0:1, :], ones_f[:, 0:1], red[:, 0:4],
                start=True, stop=True,
            )
            out_sb = stats.tile([P, 8], _F32)
            nc.vector.tensor_scalar_add(out_sb[0:1, 0:4], psum_fin[0:1, :], 0.0)
            nc.vector.tensor_reduce(out_sb[0:1, 4:5], psum_cnt[0:1, :], AX.X, OP.add)
            nc.sync.dma_start(out_d[:], out_sb[0:1, 0:5])

    # Bacc pipeline: splits multi-wait sync (TRN2 allows 1 wait/inst),
    # lowers extended-ISA .instr bytes, register allocation, etc.
    nc.compile()
    return nc


def kernel(class_output, reg_output, class_target, reg_target, class_weights):
    global LAST_RESULTS
    nc = _build_nc()

    def shards(a):
        a = np.ascontiguousarray(np.asarray(a, dtype=np.float32))
        return [
            a[c * NSHARD : (c + 1) * NSHARD].reshape(NT, P, F) for c in range(NCORES)
        ]

    ps = shards(class_output)
    ts = shards(class_target)
    ros = shards(reg_output)
    rts = shards(reg_target)
    in_maps = [
        {"p": ps[c], "t": ts[c], "ro": ros[c], "rt": rts[c]} for c in range(NCORES)
    ]

    res = run_bass_kernel_spmd(nc, in_maps, core_ids=list(range(NCORES)))
    LAST_RESULTS = res

    parts = np.stack([np.asarray(res.results[c]["out"][0]) for c in range(NCORES)])
    tot = parts.sum(axis=0, dtype=np.float64)
    s_tlq, s_lq, s_sq, s_tsq, s_t = tot

    w0 = float(np.asarray(class_weights)[0, 0])
    w1 = float(np.asarray(class_weights)[0, 1])
    # s_lq/s_tlq are sums of ln(q^2) = 2*ln(q)
    class_loss = -(w1 * s_tlq + w0 * (s_lq - s_tlq)) / (2.0 * N)
    cnt = N - s_t
    reg_loss = ((s_sq - s_tsq) / cnt) if cnt > 0 else 0.0
    return np.float32(0.5 * class_loss + 0.5 * reg_loss)


# revision 10
# speedup vs baseline: 1.1184x; 1.1184x over previous
"""Weighted-BCE + masked-MSE loss on 8 Trainium2 cores (pure data parallel).

Math (t in {0,1} exactly): let q = |1 - p - t|  (= p when t=1, 1-p when t=0).
  ln(q^2) = 2*ln(q), so one Square+Ln chain replaces both BCE logs:
    class_sum = -[w1*S_tlq + w0*(S_lq - S_tlq)] / 2,
      S_lq  = sum ln(q^2)   (free via ACT accum),
      S_tlq = sum t*ln(q^2) (one DVE dot)
  masked sq: sum (1-t)*dd^2 = S_sq - S_tsq,  dd = ro - rt
  cnt_zeros = N - S_t  (PE matmul ones.T @ t)
Each core reduces its shard to 5 scalars; host combines and applies weights.

Engine mix per tile (DMA is the bottleneck; everything else has slack):
  DMA : p,t,rt dispatched on SP HWDGE; ro on ACT HWDGE
  Pool: s = p + t (its one op)
  ACT : u2 = Square(1-s); lq2 = Ln(u2) [+accum S_lq]; sq = Square(dd) [+accum S_sq]
  DVE : dd = ro - rt; two tensor_tensor_reduce dots (t.lq2, t.sq)
  PE  : count = ones.T @ t accumulated in PSUM; final partition reduce
"""

import os
import sys

for _p in ("/opt/trn_rl_repo", "/root/.axon_site/_ro/trn_rl_repo"):
    if os.path.isdir(_p) and _p not in sys.path:
        sys.path.insert(0, _p)

import numpy as np

import concourse.bacc as bacc
import concourse.mybir as mybir
from concourse import tile
from concourse.bass_utils import run_bass_kernel_spmd

N = 16777216
NCORES = 8
NSHARD = N // NCORES  # 2097152
P = 128
F = 2048
NT = NSHARD // (P * F)  # 8

_F32 = mybir.dt.float32
_BF16 = mybir.dt.bfloat16

LAST_RESULTS = None  # test harness peeks at exec_time_ns / trace path


def _build_nc():
    AF = mybir.ActivationFunctionType
    OP = mybir.AluOpType
    AX = mybir.AxisListType

    nc = bacc.Bacc(
        "TRN2", target_bir_lowering=False, debug=False, num_devices=NCORES
    )
    p_d = nc.dram_tensor("p", [NT, P, F], _F32, kind="ExternalInput")
    t_d = nc.dram_tensor("t", [NT, P, F], _F32, kind="ExternalInput")
    ro_d = nc.dram_tensor("ro", [NT, P, F], _F32, kind="ExternalInput")
    rt_d = nc.dram_tensor("rt", [NT, P, F], _F32, kind="ExternalInput")
    out_d = nc.dram_tensor("out", [1, 5], _F32, kind="ExternalOutput")

    with tile.TileContext(nc) as tc:
        with (
            tc.tile_pool(name="io", bufs=5) as io,
            tc.tile_pool(name="work", bufs=2) as work,
            tc.tile_pool(name="stats", bufs=1) as stats,
            tc.tile_pool(name="psum", bufs=1, space="PSUM") as psum,
        ):
            acc_tlq = stats.tile([P, NT], _F32)  # sum t*ln(q^2) per tile col
            acc_lq = stats.tile([P, NT], _F32)  # sum ln(q^2)
            acc_sq = stats.tile([P, NT], _F32)  # sum (ro-rt)^2
            acc_tsq = stats.tile([P, NT], _F32)  # sum t*(ro-rt)^2

            ones_f = stats.tile([P, 1], _F32)
            nc.vector.memset(ones_f[:], 1.0)
            ones_bf = stats.tile([P, 1], _BF16)
            nc.vector.memset(ones_bf[:], 1.0)
            junk = stats.tile([P, F], _BF16)  # dead dot output, reused

            psum_cnt = psum.tile([1, 512], _F32)
            NCHUNK = F // 512

            for i in range(NT):
                tp = io.tile([P, F], _F32, tag="p")
                tt = io.tile([P, F], _BF16, tag="t")
                tro = io.tile([P, F], _F32, tag="ro")
                trt = io.tile([P, F], _F32, tag="rt")
                nc.sync.dma_start(tp[:], p_d[i, :, :])
                nc.gpsimd.dma_start(tt[:], t_d[i, :, :])  # SWDGE f32->bf16 cast
                nc.scalar.dma_start(tro[:], ro_d[i, :, :])
                nc.scalar.dma_start(trt[:], rt_d[i, :, :])

                # Pool: s = p + t (its one op)
                s = work.tile([P, F], _F32, tag="s")
                nc.gpsimd.tensor_add(s[:], tp[:], tt[:])

                # ACT: u2 = (1-s)^2 = q^2 ; lq2 = ln(q^2) with free accum
                u2 = work.tile([P, F], _BF16, tag="u2")
                nc.scalar.activation(u2[:], s[:], AF.Square, bias=1.0, scale=-1.0)
                lq2 = work.tile([P, F], _BF16, tag="lq2")
                nc.scalar.activation(
                    lq2[:], u2[:], AF.Ln, accum_out=acc_lq[:, i : i + 1]
                )

                # DVE: dd = ro - rt; dot t.lq2
                dd = work.tile([P, F], _BF16, tag="dd")
                nc.vector.tensor_sub(dd[:], tro[:], trt[:])
                nc.vector.scalar_tensor_tensor(
                    junk[:], tt[:], 1.0, lq2[:],
                    OP.mult, OP.mult, accum_out=acc_tlq[:, i : i + 1],
                )

                # ACT: sq = dd^2 with free accum; DVE: dot t.sq
                sq = work.tile([P, F], _BF16, tag="sq")
                nc.scalar.activation(
                    sq[:], dd[:], AF.Square, accum_out=acc_sq[:, i : i + 1]
                )
                nc.vector.scalar_tensor_tensor(
                    junk[:], tt[:], 1.0, sq[:],
                    OP.mult, OP.mult, accum_out=acc_tsq[:, i : i + 1],
                )

                # PE: accumulate column-sums of t into psum_cnt
                for c in range(NCHUNK):
                    nc.tensor.matmul(
                        psum_cnt[0:1, :],
                        ones_bf[:, 0:1],
                        tt[:, c * 512 : (c + 1) * 512],
                        start=(i == 0 and c == 0),
                        stop=(i == NT - 1 and c == NCHUNK - 1),
                    )

            # Fold per-tile partials into out[1,5]
            red = stats.tile([P, 4], _F32)
            for j, acc in enumerate((acc_tlq, acc_lq, acc_sq, acc_tsq)):
                nc.vector.tensor_reduce(red[:, j : j + 1], acc[:], AX.X, OP.add)
            psum_fin = psum.tile([1, 4], _F32)
            nc.tensor.matmul(
                psum_fin[0:1, :], ones_f[:, 0:1], red[:, 0:4],
                start=True, stop=True,
            )
            out_sb = stats.tile([P, 8], _F32)
            nc.vector.tensor_scalar_add(out_sb[0:1, 0:4], psum_fin[0:1, :], 0.0)
            nc.vector.tensor_reduce(out_sb[0:1, 4:5], psum_cnt[0:1, :], AX.X, OP.add)
            nc.sync.dma_start(out_d[:], out_sb[0:1, 0:5])

    # Bacc pipeline: splits multi-wait sync (TRN2 allows 1 wait/inst),
    # lowers extended-ISA .instr bytes, register allocation, etc.
    nc.compile()
    return nc


def kernel(class_output, reg_output, class_target, reg_target, class_weights):
    global LAST_RESULTS
    nc = _build_nc()

    def shards(a):
        a = np.ascontiguousarray(np.asarray(a, dtype=np.float32))
        return [
            a[c * NSHARD : (c + 1) * NSHARD].reshape(NT, P, F) for c in range(NCORES)
        ]

    ps = shards(class_output)
    ts = shards(class_target)
    ros = shards(reg_output)
    rts = shards(reg_target)
    in_maps = [
        {"p": ps[c], "t": ts[c], "ro": ros[c], "rt": rts[c]} for c in range(NCORES)
    ]

    res = run_bass_kernel_spmd(nc, in_maps, core_ids=list(range(NCORES)))
    LAST_RESULTS = res

    parts = np.stack([np.asarray(res.results[c]["out"][0]) for c in range(NCORES)])
    tot = parts.sum(axis=0, dtype=np.float64)
    s_tlq, s_lq, s_sq, s_tsq, s_t = tot

    w0 = float(np.asarray(class_weights)[0, 0])
    w1 = float(np.asarray(class_weights)[0, 1])
    # s_lq/s_tlq are sums of ln(q^2) = 2*ln(q)
    class_loss = -(w1 * s_tlq + w0 * (s_lq - s_tlq)) / (2.0 * N)
    cnt = N - s_t
    reg_loss = ((s_sq - s_tsq) / cnt) if cnt > 0 else 0.0
    return np.float32(0.5 * class_loss + 0.5 * reg_loss)
